# revision 21
# baseline (speedup 1.0000x reference)
"""Trainium2 Bass kernel for nn_CoBertCRF: token scores + CRF loss + Viterbi decode.

Self-contained: builds one SPMD Bass/Tile program, runs it on 8 NeuronCores via
run_bass_kernel_spmd, assembles full outputs on host.

Sharding: batch-parallel matmul (16 batch rows/core); CRF scans are sequential
per-timestep, so the gathered token_scores are exchanged chunk-wise via
AllGather collectives and the four scan chains (viterbi fwd/bwd, partition-sum
fwd/bwd) run on dedicated cores, pipelined behind the matmul production.
"""

import json
import numpy as np

import concourse.bass as bass
import concourse.mybir as mybir
import concourse.tile as tile
from concourse.bass import ds
from concourse.bass_utils import run_bass_kernel_spmd

fp32 = mybir.dt.float32
i32 = mybir.dt.int32
AF = mybir.ActivationFunctionType
OP = mybir.AluOpType
AX = mybir.AxisListType

B, S, H, T = 128, 512, 1024, 13
NC = 8
BL = B // NC            # batch rows per core
NCH = 8                 # sequence chunks for the exchange
CS = S // NCH           # 64 timesteps per chunk
TT = T * T              # 169
# produce chunks from both sequence ends so fwd and bwd chains both start early
SWEEP_ORDER = [0, 7, 1, 6, 2, 5, 3, 4]
RENORM_EVERY = 4
BIG = 1.0e9


# ---------------------------------------------------------------- birfix ----
# This container's walrus accepts at most ONE sync wait / update per
# instruction; Tile emits several. Split extras onto same-engine Drains.
def _carrier(name, engine, wait=None, update=None):
    si = {"on_wait": [wait] if wait is not None else [],
          "on_update": [update] if update is not None else []}
    return {"name": name, "opcode": "Drain", "engine": engine, "ins": [],
            "outs": [], "is_reset_sema": False, "sync_info": si}


def _split_multiwaits(bir_bytes: bytes) -> bytes:
    m = json.loads(bir_bytes)
    for f in m.get("functions", []):
        for b in f.get("blocks", []):
            out = []
            for ins in b.get("instructions", []):
                si = ins.get("sync_info") or {}
                waits = si.get("on_wait") or []
                updates = si.get("on_update") or []
                eng = ins.get("engine")
                pre, post = [], []
                if len(waits) > 1:
                    for i, w in enumerate(waits[:-1]):
                        pre.append(_carrier(f"{ins['name']}-w{i}", eng, wait=w))
                    si["on_wait"] = waits[-1:]
                if len(updates) > 1:
                    for i, u in enumerate(updates[1:]):
                        post.append(_carrier(f"{ins['name']}-u{i}", eng, update=u))
                    si["on_update"] = updates[:1]
                out.extend(pre)
                out.append(ins)
                out.extend(post)
            b["instructions"] = out
    return json.dumps(m).encode()


def _patch_nc(nc):
    orig = nc.to_json_bytes

    def patched(*a, **k):
        return _split_multiwaits(orig(*a, **k))

    nc.to_json_bytes = patched
    return nc


# ---------------------------------------------------------------- program ----
def build_program():
    nc = bass.Bass(num_devices=NC)

    hid_in = nc.dram_tensor("hid", [BL, S, H], fp32, kind="ExternalInput")
    wT_in = nc.dram_tensor("wT", [H, T], fp32, kind="ExternalInput")
    b_in = nc.dram_tensor("bvec", [T], fp32, kind="ExternalInput")
    st_in = nc.dram_tensor("st", [T], fp32, kind="ExternalInput")
    en_in = nc.dram_tensor("en", [T], fp32, kind="ExternalInput")
    tr_in = nc.dram_tensor("tr", [T, T], fp32, kind="ExternalInput")
    trT_in = nc.dram_tensor("trT", [T, T], fp32, kind="ExternalInput")
    lab_in = nc.dram_tensor("lab", [B, S], i32, kind="ExternalInput")
    id_in = nc.dram_tensor("ident", [128, 128], fp32, kind="ExternalInput")
    io_in = nc.dram_tensor("iota13", [1, T], fp32, kind="ExternalInput")
    on_in = nc.dram_tensor("ones128", [128, 1], fp32, kind="ExternalInput")

    ts_out = nc.dram_tensor("ts_out", [BL, S, T], fp32, kind="ExternalOutput")
    tg_out = nc.dram_tensor("tags_half", [B, S // 2], i32, kind="ExternalOutput")
    ls_out = nc.dram_tensor("loss", [1, 12], fp32, kind="ExternalOutput")
    db_out = nc.dram_tensor("dbg", [128, 48], fp32, kind="ExternalOutput")

    HALF = (S // 2) * T  # 3328

    with tile.TileContext(nc) as tc:
        pid = nc.partition_id()
        with (
            tc.tile_pool(name="const", bufs=1) as cpool,
            tc.tile_pool(name="big", bufs=1) as bigp,
            tc.tile_pool(name="hid", bufs=3) as hidp,
            tc.tile_pool(name="hidT", bufs=3) as hidTp,
            tc.tile_pool(name="tsT", bufs=2) as tsTp,
            tc.tile_pool(name="tss", bufs=3) as tssp,
            tc.tile_pool(name="chain", bufs=2) as chp,
            tc.tile_pool(name="small", bufs=2) as smp,
            tc.tile_pool(name="fin", bufs=1) as finp,
            tc.tile_pool(name="psT", bufs=2, space="PSUM") as psTp,
            tc.tile_pool(name="psTS", bufs=2, space="PSUM") as psTSp,
            tc.tile_pool(name="psB", bufs=2, space="PSUM") as psBp,
            tc.tile_pool(name="psC", bufs=1, space="PSUM") as psCp,
            tc.tile_pool(name="psN", bufs=1, space="PSUM") as psNp,
            tc.tile_pool(name="dram", bufs=1, space="DRAM") as dram,
        ):
            # ---------------- constants ----------------
            ident = cpool.tile([128, 128], fp32, tag="ident")
            nc.sync.dma_start(ident[:], id_in[:])
            wT_sb = cpool.tile([128, 8 * T], fp32, tag="wT")
            for hc in range(8):
                nc.sync.dma_start(wT_sb[:, ds(hc * T, T)],
                                  wT_in[ds(hc * 128, 128), :])
            b_col = cpool.tile([T, 1], fp32, tag="bcol")
            nc.sync.dma_start(b_col[:], b_in[:].unsqueeze(1))
            ones_col = cpool.tile([128, 1], fp32, tag="ones")
            nc.sync.dma_start(ones_col[:], on_in[:])
            tr_nat = cpool.tile([T, T], fp32, tag="trnat")
            nc.sync.dma_start(tr_nat[:], tr_in[:])

            # single-row copies of the small vectors for PE replication
            row = cpool.tile([1, 2 * TT + 3 * T + T], fp32, tag="rowbuf")
            # layout: [trT 169][tr 169][start 13][end 13][iota 13]
            nc.sync.dma_start(row[:, ds(0, TT)],
                              trT_in[:].rearrange("a b -> (a b)").unsqueeze(0))
            nc.sync.dma_start(row[:, ds(TT, TT)],
                              tr_in[:].rearrange("a b -> (a b)").unsqueeze(0))
            nc.sync.dma_start(row[:, ds(2 * TT, T)], st_in[:].unsqueeze(0))
            nc.sync.dma_start(row[:, ds(2 * TT + T, T)], en_in[:].unsqueeze(0))
            nc.sync.dma_start(row[:, ds(2 * TT + 2 * T, T)], io_in[:])

            ones_row = cpool.tile([1, 128], fp32, tag="onesrow")
            nc.vector.memset(ones_row[:], 1.0)

            def replicate(src_ap, n, tag):
                ps = psTp.tile([128, 512], fp32, tag="pst")
                nc.tensor.matmul(ps[:, 0:n], ones_row[:], src_ap, start=True, stop=True)
                t_ = cpool.tile([128, n], fp32, tag=tag)
                nc.scalar.activation(t_[:], ps[:, 0:n], AF.Copy)
                return t_

            trT_rep = replicate(row[:, ds(0, TT)], TT, "trTrep")      # [t'*13+t] = trans[t,t']
            tr_rep = replicate(row[:, ds(TT, TT)], TT, "trrep")       # [t*13+t''] = trans[t,t'']
            st_rep = replicate(row[:, ds(2 * TT, T)], T, "strep")
            en_rep = replicate(row[:, ds(2 * TT + T, T)], T, "enrep")
            iota_rep = replicate(row[:, ds(2 * TT + 2 * T, T)], T, "iorep")

            eTrT_rep = cpool.tile([128, TT], fp32, tag="eTrT")
            nc.scalar.activation(eTrT_rep[:], trT_rep[:], AF.Exp)
            eTr_rep = cpool.tile([128, TT], fp32, tag="eTr")
            nc.scalar.activation(eTr_rep[:], tr_rep[:], AF.Exp)

            # big persistent buffers
            embuf = bigp.tile([128, S * T], fp32, tag="embuf")    # token scores, all b
            histbuf = bigp.tile([128, S * T], fp32, tag="hist")   # alpha/beta/eEm per role
            recvbuf = bigp.tile([128, HALF], fp32, tag="recv")    # decode exchange

            # labels (full, all cores): int -> f32
            lab_i = bigp.tile([128, S], i32, tag="labi")
            nc.sync.dma_start(lab_i[:], lab_in[:])
            lab_f = bigp.tile([128, S], fp32, tag="labf")
            nc.vector.tensor_copy(lab_f[:], lab_i[:])

            # dram staging / collective tiles
            stage = [dram.tile([BL, CS, T], fp32, tag=f"stg{c}", name=f"stg{c}") for c in range(NCH)]
            gath = [dram.tile([B, CS, T], fp32, tag=f"gth{c}", name=f"gth{c}") for c in range(NCH)]
            sig_in = dram.tile([128, 16], fp32, tag="sigin")
            sig_out = dram.tile([128, 16], fp32, tag="sigout")
            bh_in = dram.tile([128, HALF], fp32, tag="bhin")
            bh_out = dram.tile([128, HALF], fp32, tag="bhout")
            ah_in = dram.tile([128, HALF], fp32, tag="ahin")
            ah_out = dram.tile([128, HALF], fp32, tag="ahout")
            fin_in = dram.tile([1, 8], fp32, tag="finin")
            fin_out = dram.tile([1, 8], fp32, tag="finout")

            groups = [list(range(NC))]

            # single shared [1,1] PSUM accumulator: numerator partials on
            # cores 4-7, Z-sum on core 3 (all writers are PE; same-engine
            # FIFO keeps per-core accumulation groups intact)
            acc11 = psNp.tile([1, 1], fp32, tag="psn")

            # ---------------- production: matmul + exchange ----------------
            for c in SWEEP_ORDER:
                s0 = c * CS
                for i in range(BL // 2):
                    ba, bb = 2 * i, 2 * i + 1
                    hid = hidp.tile([128, H], fp32, tag="hidt")
                    nc.sync.dma_start(hid[0:CS, :], hid_in[ba, ds(s0, CS), :])
                    nc.sync.dma_start(hid[CS:128, :], hid_in[bb, ds(s0, CS), :])
                    hidT = hidTp.tile([128, H], fp32, tag="hidTt")
                    for hc in range(8):
                        pt = psTp.tile([128, 128], fp32, tag="pst")
                        nc.tensor.transpose(pt[:], hid[:, ds(hc * 128, 128)], ident[:])
                        nc.scalar.activation(hidT[:, ds(hc * 128, 128)], pt[:], AF.Copy)
                    pts = psTSp.tile([T, 128], fp32, tag="psts")
                    for hc in range(8):
                        nc.tensor.matmul(pts[:], wT_sb[:, ds(hc * T, T)],
                                         hidT[:, ds(hc * 128, 128)],
                                         start=(hc == 0), stop=(hc == 7))
                    tsT = tsTp.tile([T, 128], fp32, tag="tsTt")
                    nc.scalar.activation(tsT[:], pts[:], AF.Identity, bias=b_col[:])
                    pb = psBp.tile([128, T], fp32, tag="psb")
                    nc.tensor.transpose(pb[:], tsT[:], ident[0:T, 0:T])
                    tss = tssp.tile([128, T], fp32, tag="tsst")
                    nc.scalar.activation(tss[:], pb[:], AF.Copy)
                    nc.gpsimd.dma_start(ts_out[ba, ds(s0, CS), :], tss[0:CS, :])
                    nc.gpsimd.dma_start(ts_out[bb, ds(s0, CS), :], tss[CS:128, :])
                    nc.gpsimd.dma_start(stage[c][ba, :, :], tss[0:CS, :])
                    nc.gpsimd.dma_start(stage[c][bb, :, :], tss[CS:128, :])
                nc.gpsimd.collective_compute(
                    "AllGather", OP.bypass, replica_groups=groups,
                    ins=[stage[c].opt()], outs=[gath[c].opt()])
                nc.sync.dma_start(embuf[:, ds(s0 * T, CS * T)],
                                  gath[c][:].rearrange("b s t -> b (s t)"))
                # sum-chain cores exponentiate their half as chunks arrive
                if c < NCH // 2:
                    with tc.If(pid == 2):
                        nc.scalar.activation(histbuf[:, ds(s0 * T, CS * T)],
                                             embuf[:, ds(s0 * T, CS * T)], AF.Exp)
                else:
                    with tc.If(pid == 3):
                        nc.scalar.activation(histbuf[:, ds(s0 * T, CS * T)],
                                             embuf[:, ds(s0 * T, CS * T)], AF.Exp)

            # ---------------- chain helpers ----------------
            def em_col(s):
                return embuf[:, ds(s * T, T)]

            def hist_col(s):
                return histbuf[:, ds(s * T, T)]

            def bcast(ap13):
                return ap13.unsqueeze(1).broadcast_to((128, T, T))

            # ---------------- core 0: viterbi forward (alpha) ----------------
            with tc.If(pid == 0):
                nc.vector.tensor_tensor(out=hist_col(0), in0=st_rep[:],
                                        in1=em_col(0), op=OP.add)
                for s in range(1, S):
                    cand = chp.tile([128, TT], fp32, tag="cand")
                    nc.vector.tensor_tensor(out=cand[:], in0=bcast(hist_col(s - 1)),
                                            in1=trT_rep[:], op=OP.add)
                    mx = chp.tile([128, T], fp32, tag="mx")
                    nc.vector.tensor_reduce(
                        out=mx[:], in_=cand[:].rearrange("p (a b) -> p a b", b=T),
                        axis=AX.X, op=OP.max)
                    nc.vector.tensor_tensor(out=hist_col(s), in0=mx[:],
                                            in1=em_col(s), op=OP.add)

            # ---------------- core 1: viterbi backward (beta) ----------------
            with tc.If(pid == 1):
                nc.vector.tensor_copy(hist_col(S - 1), en_rep[:])
                for s in range(S - 2, -1, -1):
                    u = chp.tile([128, T], fp32, tag="ub")
                    nc.vector.tensor_tensor(out=u[:], in0=hist_col(s + 1),
                                            in1=em_col(s + 1), op=OP.add)
                    cand = chp.tile([128, TT], fp32, tag="cand")
                    nc.vector.tensor_tensor(out=cand[:], in0=bcast(u[:]),
                                            in1=tr_rep[:], op=OP.add)
                    nc.vector.tensor_reduce(
                        out=hist_col(s), in_=cand[:].rearrange("p (a b) -> p a b", b=T),
                        axis=AX.X, op=OP.max)

            # ---------------- cores 2/3: partition sum halves ----------------
            def sum_chain(fwd: bool):
                # exp-space scaled forward/backward algorithm
                sig = smp.tile([128, T], fp32, tag="sig")
                acc = smp.tile([128, 1], fp32, tag="acc")
                nc.vector.memset(acc[:], 0.0)
                if fwd:
                    t0 = smp.tile([128, T], fp32, tag="sgt")
                    nc.vector.tensor_tensor(out=t0[:], in0=st_rep[:],
                                            in1=em_col(0), op=OP.add)
                    nc.scalar.activation(sig[:], t0[:], AF.Exp)
                    srange = range(1, S // 2)
                    emat = eTrT_rep
                else:
                    nc.scalar.activation(sig[:], en_rep[:], AF.Exp)
                    srange = range(S - 2, S // 2 - 2, -1)
                    emat = eTr_rep
                for k, s in enumerate(srange):
                    if fwd:
                        w = sig
                    else:
                        w = smp.tile([128, T], fp32, tag="w890")
                        nc.vector.tensor_tensor(out=w[:], in0=sig[:],
                                                in1=hist_col(s + 1), op=OP.mult)
                    tmp = chp.tile([128, TT], fp32, tag="cand")
                    nc.vector.tensor_tensor(out=tmp[:], in0=bcast(w[:]),
                                            in1=emat[:], op=OP.mult)
                    red = smp.tile([128, T], fp32, tag="red")
                    nc.vector.tensor_reduce(
                        out=red[:], in_=tmp[:].rearrange("p (a b) -> p a b", b=T),
                        axis=AX.X, op=OP.add)
                    nsig = smp.tile([128, T], fp32, tag="sig")
                    if fwd:
                        nc.vector.tensor_tensor(out=nsig[:], in0=red[:],
                                                in1=hist_col(s), op=OP.mult)
                    else:
                        nc.vector.tensor_copy(nsig[:], red[:])
                    sig = nsig
                    if (k % RENORM_EVERY) == RENORM_EVERY - 1:
                        m = smp.tile([128, 1], fp32, tag="rm")
                        nc.vector.tensor_reduce(out=m[:], in_=sig[:], axis=AX.X,
                                                op=OP.max)
                        r = smp.tile([128, 1], fp32, tag="rr")
                        nc.vector.reciprocal(r[:], m[:])
                        nsig2 = smp.tile([128, T], fp32, tag="sig")
                        nc.vector.tensor_scalar_mul(nsig2[:], sig[:], r[:])
                        sig = nsig2
                        lm = smp.tile([128, 1], fp32, tag="lm")
                        nc.scalar.activation(lm[:], m[:], AF.Ln)
                        acc2 = smp.tile([128, 1], fp32, tag="acc")
                        nc.vector.tensor_tensor(out=acc2[:], in0=acc[:], in1=lm[:],
                                                op=OP.add)
                        acc = acc2
                return sig, acc

            pack = finp.tile([128, 16], fp32, tag="pack")
            nc.vector.memset(pack[:], 0.0)
            with tc.If(pid == 2):
                sigf, accf = sum_chain(fwd=True)
                nc.vector.tensor_copy(pack[:, 0:T], sigf[:])
                nc.vector.tensor_copy(pack[:, T:T + 1], accf[:])
            nc.gpsimd.dma_start(sig_in[:], pack[:])

            nc.gpsimd.collective_compute(
                "AllReduce", OP.add, replica_groups=groups,
                ins=[sig_in.opt()], outs=[sig_out.opt()])
            srecv = finp.tile([128, 16], fp32, tag="srecv")
            nc.sync.dma_start(srecv[:], sig_out[:])

            dbt = finp.tile([128, 48], fp32, tag="dbt")
            nc.vector.memset(dbt[:], 0.0)
            nc.vector.tensor_copy(dbt[:, 0:16], srecv[:])
            # core 3: finish Z = log(sum_t sigma*gamma) + accs, all-b sum
            with tc.If(pid == 3):
                sigb, accb = sum_chain(fwd=False)
                nc.vector.tensor_copy(dbt[:, 16:16 + T], sigb[:])
                nc.vector.tensor_copy(dbt[:, 30:31], accb[:])
                dot = smp.tile([128, T], fp32, tag="dot")
                nc.vector.tensor_tensor(out=dot[:], in0=srecv[:, 0:T], in1=sigb[:],
                                        op=OP.mult)
                zred = smp.tile([128, 1], fp32, tag="zred")
                nc.vector.tensor_reduce(out=zred[:], in_=dot[:], axis=AX.X, op=OP.add)
                zlog = smp.tile([128, 1], fp32, tag="zlog")
                nc.scalar.activation(zlog[:], zred[:], AF.Ln)
                z1 = smp.tile([128, 1], fp32, tag="z1")
                nc.vector.tensor_tensor(out=z1[:], in0=zlog[:], in1=srecv[:, T:T + 1],
                                        op=OP.add)
                z2 = smp.tile([128, 1], fp32, tag="z2")
                nc.vector.tensor_tensor(out=z2[:], in0=z1[:], in1=accb[:], op=OP.add)
                nc.tensor.matmul(acc11[:], ones_col[:], z2[:], start=True, stop=True)
                nc.vector.tensor_copy(dbt[:, 31:32], zred[:])
                nc.vector.tensor_copy(dbt[:, 32:33], z2[:])

            # ---------------- cores 4..7: numerator quarters ----------------
            QS = S // 4  # 128
            for q in range(4):
                with tc.If(pid == 4 + q):
                    sq = q * QS
                    psc = psCp.tile([T, T], fp32, tag="psc")
                    npair = QS if q < 3 else QS - 1
                    oh_prev = None
                    for sl in range(npair + 1):
                        s = sq + sl
                        if s >= S:
                            break
                        oh = tssp.tile([128, T], fp32, tag="oh")
                        nc.vector.tensor_tensor(
                            out=oh[:], in0=iota_rep[:],
                            in1=lab_f[:, ds(s, 1)].broadcast_to((128, T)),
                            op=OP.is_equal)
                        if oh_prev is not None:
                            nc.tensor.matmul(psc[:], oh_prev[:], oh[:],
                                             start=(sl == 1), stop=(sl == npair))
                        oh_prev = oh
                        if q == 0 and sl == 0:
                            # start_trans[l0] + em[:,0,l0]
                            se = smp.tile([128, T], fp32, tag="se")
                            nc.vector.tensor_tensor(out=se[:], in0=st_rep[:],
                                                    in1=em_col(0), op=OP.add)
                            g0 = smp.tile([128, T], fp32, tag="g0")
                            nc.vector.tensor_tensor(out=g0[:], in0=se[:], in1=oh[:],
                                                    op=OP.mult)
                            g0r = smp.tile([128, 1], fp32, tag="g0r")
                            nc.vector.tensor_reduce(out=g0r[:], in_=g0[:], axis=AX.X,
                                                    op=OP.add)
                            nc.tensor.matmul(acc11[:], ones_col[:], g0r[:],
                                             start=True, stop=False)
                        if q == 3 and s == S - 1:
                            ge = smp.tile([128, T], fp32, tag="ge")
                            nc.vector.tensor_tensor(out=ge[:], in0=en_rep[:],
                                                    in1=oh[:], op=OP.mult)
                            ger = smp.tile([128, 1], fp32, tag="ger")
                            nc.vector.tensor_reduce(out=ger[:], in_=ge[:], axis=AX.X,
                                                    op=OP.add)
                            nc.tensor.matmul(acc11[:], ones_col[:], ger[:],
                                             start=True, stop=False)
                    # bulk em_tag over this quarter (s>=1 only for q==0)
                    lo = sq if q > 0 else 1
                    n_s = QS - (lo - sq)
                    eq = bigp.tile([128, QS * T], fp32, tag="eqbig")
                    nc.vector.tensor_tensor(
                        out=eq[:, 0:n_s * T].rearrange("p (a b) -> p a b", b=T),
                        in0=iota_rep[:].unsqueeze(1).broadcast_to((128, n_s, T)),
                        in1=lab_f[:, ds(lo, n_s)].unsqueeze(2).broadcast_to((128, n_s, T)),
                        op=OP.is_equal)
                    emt = bigp.tile([128, QS * T], fp32, tag="emtb")
                    nc.vector.tensor_tensor(out=emt[:, 0:n_s * T],
                                            in0=eq[:, 0:n_s * T],
                                            in1=embuf[:, ds(lo * T, n_s * T)],
                                            op=OP.mult)
                    emr = smp.tile([128, 1], fp32, tag="emr")
                    nc.vector.tensor_reduce(
                        out=emr[:], in_=emt[:, 0:n_s * T].rearrange(
                            "p (a b) -> p a b", b=T),
                        axis=AX.XY, op=OP.add)
                    nc.tensor.matmul(acc11[:], ones_col[:], emr[:],
                                     start=(q in (1, 2)), stop=False)
                    # pair transition sum via count matrix
                    csb = smp.tile([T, T], fp32, tag="csb")
                    nc.vector.tensor_copy(csb[:], psc[:])
                    cw = smp.tile([T, T], fp32, tag="cw")
                    nc.vector.tensor_tensor(out=cw[:], in0=csb[:], in1=tr_nat[:],
                                            op=OP.mult)
                    cwr = smp.tile([T, 1], fp32, tag="cwr")
                    nc.vector.tensor_reduce(out=cwr[:], in_=cw[:], axis=AX.X, op=OP.add)
                    nc.tensor.matmul(acc11[:], ones_col[0:T, :], cwr[:],
                                     start=False, stop=True)

            # ---------------- alpha/beta exchange for decode ----------------
            # recvbuf doubles as the zero-padded outgoing stage, then receives.
            nc.vector.memset(recvbuf[:], 0.0)
            with tc.If(pid == 1):
                nc.vector.tensor_copy(recvbuf[:], histbuf[:, 0:HALF])
            nc.gpsimd.dma_start(bh_in[:], recvbuf[:])
            nc.gpsimd.collective_compute(
                "AllReduce", OP.add, replica_groups=groups,
                ins=[bh_in.opt()], outs=[bh_out.opt()])
            nc.vector.memset(recvbuf[:], 0.0)
            with tc.If(pid == 0):
                nc.vector.tensor_copy(recvbuf[:], histbuf[:, HALF:2 * HALF])
            nc.gpsimd.dma_start(ah_in[:], recvbuf[:])
            nc.gpsimd.collective_compute(
                "AllReduce", OP.add, replica_groups=groups,
                ins=[ah_in.opt()], outs=[ah_out.opt()])

            # ---------------- decode on cores 0/1 ----------------
            def decode(alpha_region, beta_region):
                SH = S // 2
                sc = bigp.tile([128, HALF], fp32, tag="score")
                nc.vector.tensor_tensor(out=sc[:], in0=alpha_region, in1=beta_region,
                                        op=OP.add)
                mr = bigp.tile([128, SH], fp32, tag="mrow")
                nc.vector.tensor_reduce(
                    out=mr[:], in_=sc[:].rearrange("p (a b) -> p a b", b=T),
                    axis=AX.X, op=OP.max)
                lt = bigp.tile([128, HALF], fp32, tag="ltb")
                nc.vector.tensor_tensor(
                    out=lt[:].rearrange("p (a b) -> p a b", b=T),
                    in0=sc[:].rearrange("p (a b) -> p a b", b=T),
                    in1=mr[:].unsqueeze(2).broadcast_to((128, SH, T)),
                    op=OP.is_lt)
                idxf = bigp.tile([128, HALF], fp32, tag="idxf")
                nc.vector.scalar_tensor_tensor(
                    out=idxf[:].rearrange("p (a b) -> p a b", b=T),
                    in0=lt[:].rearrange("p (a b) -> p a b", b=T), scalar=BIG,
                    in1=iota_rep[:].unsqueeze(1).broadcast_to((128, SH, T)),
                    op0=OP.mult, op1=OP.add)
                im = bigp.tile([128, SH], fp32, tag="imrow")
                nc.vector.tensor_reduce(
                    out=im[:], in_=idxf[:].rearrange("p (a b) -> p a b", b=T),
                    axis=AX.X, op=OP.min)
                nc.vector.tensor_copy(ti[:], im[:])

            ti = bigp.tile([128, S // 2], i32, tag="tirow")
            nc.sync.dma_start(recvbuf[:], bh_out[:])
            with tc.If(pid == 0):
                decode(histbuf[:, 0:HALF], recvbuf[:])
            nc.sync.dma_start(recvbuf[:], ah_out[:])
            with tc.If(pid == 1):
                decode(recvbuf[:], histbuf[:, HALF:2 * HALF])
            nc.gpsimd.dma_start(tg_out[:], ti[:])

            # ---------------- final loss vector ----------------
            fvec = finp.tile([1, 8], fp32, tag="fvec")
            nc.vector.memset(fvec[:], 0.0)
            for q in range(4):
                with tc.If(pid == 4 + q):
                    nc.vector.tensor_scalar_mul(fvec[:, ds(q, 1)], acc11[:], -1.0)
            with tc.If(pid == 3):
                nc.vector.tensor_copy(fvec[:, ds(4, 1)], acc11[:])
            nc.gpsimd.dma_start(fin_in[:], fvec[:])
            nc.gpsimd.collective_compute(
                "AllReduce", OP.add, replica_groups=groups,
                ins=[fin_in.opt()], outs=[fin_out.opt()])

            # ---------------- loss everywhere ----------------
            fr = finp.tile([1, 8], fp32, tag="fr")
            nc.sync.dma_start(fr[:], fin_out[:])
            fs = finp.tile([1, 1], fp32, tag="fs")
            nc.vector.tensor_reduce(out=fs[:], in_=fr[:], axis=AX.X, op=OP.add)
            fl = finp.tile([1, 12], fp32, tag="fl")
            nc.vector.memset(fl[:], 0.0)
            nc.vector.tensor_copy(fl[:, 0:8], fr[:])
            nc.vector.tensor_scalar_mul(fl[:, 8:9], fs[:], 1.0 / B)
            nc.gpsimd.dma_start(ls_out[:], fl[:])
            nc.gpsimd.dma_start(db_out[:], dbt[:])

    return _patch_nc(nc)


_CACHED = {}
_LAST_RES = None


def kernel(hidden, W, b, start_trans, end_trans, trans, labels, mask):
    hidden = np.ascontiguousarray(np.asarray(hidden, dtype=np.float32))
    W = np.asarray(W, dtype=np.float32)
    b = np.asarray(b, dtype=np.float32)
    start_trans = np.asarray(start_trans, dtype=np.float32)
    end_trans = np.asarray(end_trans, dtype=np.float32)
    trans = np.asarray(trans, dtype=np.float32)
    labels_i = np.ascontiguousarray(np.asarray(labels).astype(np.int32))
    # mask is all ones by construction (fill: ones); computation assumes it.

    if "nc" not in _CACHED:
        _CACHED["nc"] = build_program()
    nc = _CACHED["nc"]

    wT = np.ascontiguousarray(W.T)                      # [H, T]
    trT = np.ascontiguousarray(trans.T)                 # [t', t]
    ident = np.eye(128, dtype=np.float32)
    iota13 = np.arange(T, dtype=np.float32).reshape(1, T)
    ones128 = np.ones((128, 1), dtype=np.float32)

    in_maps = []
    for k in range(NC):
        in_maps.append({
            "hid": np.ascontiguousarray(hidden[k * BL:(k + 1) * BL]),
            "wT": wT, "bvec": b, "st": start_trans, "en": end_trans,
            "tr": trans, "trT": trT, "lab": labels_i,
            "ident": ident, "iota13": iota13, "ones128": ones128,
        })

    import os
    trace = bool(int(os.environ.get("CRF_TRACE", "0")))
    res = run_bass_kernel_spmd(nc, in_maps, core_ids=list(range(NC)),
                               trace=trace)
    if trace:
        print("exec_time_ns:", res.exec_time_ns,
              "mean:", res.mean_exec_time_ns,
              "trace:", (res.instructions_and_trace or (None, None))[1])
    outs = res.results
    global _LAST_RES
    _LAST_RES = outs

    token_scores = np.concatenate([outs[k]["ts_out"] for k in range(NC)], axis=0)
    tags = np.concatenate([outs[0]["tags_half"], outs[1]["tags_half"]], axis=1)
    tags = tags.astype(np.int32)
    dbg = outs[0]["loss"].reshape(-1)
    kernel.debug_partials = dbg.copy()
    loss = np.float32(dbg[8])
    return loss, tags, token_scores
